# revision 10
# baseline (speedup 1.0000x reference)
"""Trainium2 Bass kernel for gated pair-bias attention (B=8,S=1024,D=256,H=8,DH=32).

Sharding: data-parallel over batch — core b computes batch element b entirely;
weights + pair bias replicated to all 8 cores.

Per-core math (batch index dropped):
  g     = sigmoid(q @ Wg^T + bg)                      [S, E]
  qh    = (q @ Wq^T) * DH^-0.5 ; kh = k @ Wk^T ; vh = v @ Wv^T
  s_hqk = qh_h @ kh_h^T + mask + bias_h               (mask folded host-side)
  attn  = softmax_k(s) ;  o = attn @ vh_h ;  o = g * o ;  out = o @ Wo^T

Layout strategy: every operand that a PE contraction needs with its
contraction axis on partitions is pre-transposed ON THE HOST (free) and sent
in that layout: qT/kT/vT [D,S], W*^T [D,E], Wo^T [E,D], and the pair bias
transposed per head to [H, S_k, S_q] in bf16. On-chip:
  - scores computed transposed: sT[k,q] = khT_h-slice^T @ qhT_h (K=DH=32)
  - bias added into the scores psum by a stationary-identity matmul
    (psum += I^T @ biasT_slab) — zero DVE cost, stream-rate on PE
  - exp on ACT straight PSUM->SBUF with no max subtraction (|scores| <= ~7,
    fine in f32); flash-style unnormalized oT plus row-sums in one M=64
    matmul whose stationary is [vh_h | ones] (interleaved 64-col blocks),
    dividing by sigma once at the end.
"""

import os
import sys

import numpy as np

for _p in ("/opt/trn_rl_repo", "/root/.axon_site/_ro/trn_rl_repo"):
    if os.path.isdir(_p) and _p not in sys.path:
        sys.path.append(_p)

import ml_dtypes
import concourse.bass as bass
import concourse.mybir as mybir
import concourse.tile as tile
from concourse import bacc
from concourse.bass_utils import run_bass_kernel_spmd
from concourse.masks import make_identity

S, D, E, H, DH = 1024, 256, 256, 8, 32
NCORES = 8
F32 = mybir.dt.float32
BF16 = mybir.dt.bfloat16
NORM = float(DH) ** -0.5
ST = S // 128   # 8 s-tiles
DT = D // 128   # 2 d-tiles
ET = E // 128   # 2 e-tiles
Act = mybir.ActivationFunctionType


def build_bass() -> bass.Bass:
    # Bacc (not raw Bass): its compile() runs move_matmul_waits_to_ldweights +
    # generate_event_semaphores, which split multi-semaphore waits that the
    # TRN2 instruction encodings cannot carry (walrus rejects them otherwise).
    nc = bacc.Bacc("TRN2", target_bir_lowering=False, debug=True)

    qT_d = nc.dram_tensor("qT", [D, S], BF16, kind="ExternalInput")
    kT_d = nc.dram_tensor("kT", [D, S], BF16, kind="ExternalInput")
    vT_d = nc.dram_tensor("vT", [D, S], BF16, kind="ExternalInput")
    biasT_d = nc.dram_tensor("biasT", [H, S, S], BF16, kind="ExternalInput")
    w_d = {  # all pre-transposed on host; "q" also pre-scaled by DH^-0.5
        "q": nc.dram_tensor("WqT", [D, E], BF16, kind="ExternalInput"),
        "k": nc.dram_tensor("WkT", [D, E], BF16, kind="ExternalInput"),
        "v": nc.dram_tensor("WvT", [D, E], BF16, kind="ExternalInput"),
        "g": nc.dram_tensor("WgT", [D, E], BF16, kind="ExternalInput"),
        "o": nc.dram_tensor("WoT", [E, D], BF16, kind="ExternalInput"),
    }
    bg_d = nc.dram_tensor("bg", [E], F32, kind="ExternalInput")
    out_d = nc.dram_tensor("out", [S, D], F32, kind="ExternalOutput")

    with tile.TileContext(nc) as tc:
        with (
            tc.tile_pool(name="const", bufs=1) as constp,
            tc.tile_pool(name="persist", bufs=1) as persist,
            tc.tile_pool(name="biasp", bufs=6) as biasp,
            tc.tile_pool(name="expp", bufs=3) as expp,
            tc.tile_pool(name="smallp", bufs=4) as smallp,
            tc.tile_pool(name="outp", bufs=3) as outp,
            tc.tile_pool(name="psum", bufs=2, space="PSUM") as psum,
        ):
            ident = constp.tile([128, 128], BF16)
            make_identity(nc, ident[:])
            bg_sb = constp.tile([128, ET], F32)
            bg2d = bg_d.rearrange("(a b) -> a b", b=1)
            for et in range(ET):
                nc.sync.dma_start(out=bg_sb[:, et : et + 1],
                                  in_=bg2d[et * 128 : (et + 1) * 128, :])

            WT = {}
            for nm, wd in w_d.items():
                wts = []
                for i in range(2):
                    wdt = BF16
                    wt = constp.tile([128, E], wdt, name=f"WT_{nm}{i}",
                                     tag=f"WT_{nm}{i}")
                    nc.sync.dma_start(out=wt[:], in_=wd[i * 128 : (i + 1) * 128, :])
                    wts.append(wt)
                WT[nm] = wts

            def load_T(src_d, pref):
                tiles = []
                for i in range(DT):
                    t = persist.tile([128, S], BF16, name=f"{pref}T{i}",
                                     tag=f"{pref}T{i}")
                    nc.sync.dma_start(out=t[:], in_=src_d[i * 128 : (i + 1) * 128, :])
                    tiles.append(t)
                return tiles

            qT = load_T(qT_d, "q")
            kT = load_T(kT_d, "k")
            vT = load_T(vT_d, "v")

            # ---- projections ----
            def proj_T(dst_tiles_cb, wname, xT):
                # out[e-tile, s] = W^T-slice^T @ xT, accumulated over d tiles
                for et in range(ET):
                    ps_p = psum.tile([128, S], F32, tag="ps_big", bufs=3,
                                     name=f"ps_{wname}{et}")
                    for dt in range(DT):
                        for qc in range(2):
                            nc.tensor.matmul(
                                ps_p[:, qc * 512 : (qc + 1) * 512],
                                lhsT=WT[wname][dt][:, et * 128 : (et + 1) * 128],
                                rhs=xT[dt][:, qc * 512 : (qc + 1) * 512],
                                start=(dt == 0), stop=(dt == DT - 1))
                    dst_tiles_cb(et, ps_p)

            qhT = [persist.tile([128, S], BF16, name=f"qhT{i}") for i in range(ET)]
            proj_T(lambda et, ps: nc.vector.tensor_copy(qhT[et][:], ps[:]), "q", qT)
            gateT = [persist.tile([128, S], F32, name=f"gateT{i}") for i in range(ET)]
            proj_T(lambda et, ps: nc.scalar.activation(
                gateT[et][:], ps[:], Act.Sigmoid, bias=bg_sb[:, et : et + 1]), "g", qT)
            khT = [persist.tile([128, S], BF16, name=f"khT{i}") for i in range(ET)]
            proj_T(lambda et, ps: nc.vector.tensor_copy(khT[et][:], ps[:]), "k", kT)

            # vh_aug[st]: [128, 512] with head h at cols 64h..64h+31 (= vh_h)
            # and 64h+32..64h+63 all-ones (row-sum trick); lhsT slices stay
            # contiguous per head.
            vh_aug = [persist.tile([128, 8 * 64], BF16, name=f"vh_aug{i}")
                      for i in range(ST)]
            for st in range(ST):
                nc.gpsimd.memset(
                    vh_aug[st].rearrange("p (h c) -> p h c", c=64)[:, :, DH : 2 * DH],
                    1.0)
            for st in range(ST):
                ps_v = psum.tile([128, E], F32, tag="ps_big", bufs=3, name="ps_v")
                for dt in range(DT):
                    nc.tensor.matmul(ps_v[:],
                                     lhsT=vT[dt][:, st * 128 : (st + 1) * 128],
                                     rhs=WT["v"][dt][:],
                                     start=(dt == 0), stop=(dt == DT - 1))
                nc.vector.tensor_copy(
                    vh_aug[st].rearrange("p (h c) -> p h c", c=64)[:, :, 0:DH],
                    ps_v[:].rearrange("p (h c) -> p h c", c=DH))

            # ---- attention, one head at a time ----
            o_gT = [persist.tile([128, S], BF16, name=f"o_gT{i}") for i in range(ET)]
            for h in range(H):
                et, hr = h // 4, (h % 4) * DH
                slabs = []
                for kb in range(ST):
                    bslab = biasp.tile([128, S], BF16, tag="bslab",
                                       name=f"bslab_h{h}_k{kb}")
                    nc.sync.dma_start(out=bslab[:],
                                      in_=biasT_d[h, kb * 128 : (kb + 1) * 128, :])
                    slabs.append(bslab)
                ps_o = psum.tile([64, S], F32, tag="ps_o", bufs=1)
                for kt in range(ST):
                    ps_s = psum.tile([128, S], F32, tag="ps_big", bufs=3, name="ps_s")
                    for qc in range(2):  # sT[k_tile, q] = khT_h-slice^T @ qhT_h
                        nc.tensor.matmul(
                            ps_s[:, qc * 512 : (qc + 1) * 512],
                            lhsT=khT[et][hr : hr + DH, kt * 128 : (kt + 1) * 128],
                            rhs=qhT[et][hr : hr + DH, qc * 512 : (qc + 1) * 512],
                            start=True, stop=False,
                            tile_position=(hr, 0))
                    for qc in range(2):  # += biasT slab via stationary identity
                        nc.tensor.matmul(
                            ps_s[:, qc * 512 : (qc + 1) * 512],
                            lhsT=ident[:],
                            rhs=slabs[kt][:, qc * 512 : (qc + 1) * 512],
                            start=False, stop=True)
                    expT = expp.tile([128, S], BF16, tag="expT")
                    nc.scalar.activation(expT[:], ps_s[:], Act.Exp)
                    for qc in range(2):  # oT (rows 0-31) & sigma (rows 32-63)
                        nc.tensor.matmul(
                            ps_o[:, qc * 512 : (qc + 1) * 512],
                            lhsT=vh_aug[kt][:, h * 64 : (h + 1) * 64],
                            rhs=expT[:, qc * 512 : (qc + 1) * 512],
                            start=(kt == 0), stop=(kt == ST - 1))
                # normalize + gate:  o_gT[h rows] = oT * gateT * (1/sigma)
                rsig = smallp.tile([DH, S], F32, tag="rsig")
                nc.vector.reciprocal(rsig[:], ps_o[DH : 2 * DH, :])
                tmp_o = smallp.tile([DH, S], F32, tag="tmp_o")
                nc.vector.tensor_mul(tmp_o[:], ps_o[0:DH, :],
                                     gateT[et][hr : hr + DH, :])
                nc.vector.tensor_mul(o_gT[et][hr : hr + DH, :], tmp_o[:], rsig[:])

            # ---- output projection ----
            for st in range(ST):
                ps_out = psum.tile([128, D], F32, tag="ps_big", bufs=3, name="ps_out")
                for et in range(ET):
                    nc.tensor.matmul(ps_out[:],
                                     lhsT=o_gT[et][:, st * 128 : (st + 1) * 128],
                                     rhs=WT["o"][et][:],
                                     start=(et == 0), stop=(et == ET - 1))
                o_sb = outp.tile([128, D], F32, tag="o_sb")
                nc.vector.tensor_copy(o_sb[:], ps_out[:])
                nc.sync.dma_start(out=out_d[st * 128 : (st + 1) * 128, :], in_=o_sb[:])

    nc.compile()
    return nc


_CACHED = {}


def run(inputs: dict, trace: bool = False, **spmd_kwargs):
    if "nc" not in _CACHED:
        _CACHED["nc"] = build_bass()
    nc = _CACHED["nc"]

    f32 = np.float32
    q = np.asarray(inputs["q"], dtype=f32)
    k = np.asarray(inputs["k"], dtype=f32)
    v = np.asarray(inputs["v"], dtype=f32)
    mask = np.asarray(inputs["mask"], dtype=f32)
    bias = np.asarray(inputs["bias"], dtype=f32).reshape(H, S, S)

    wqT = np.ascontiguousarray((np.asarray(inputs["Wq"], dtype=f32).T * NORM).astype(ml_dtypes.bfloat16))
    wkT = np.ascontiguousarray(np.asarray(inputs["Wk"], dtype=f32).T.astype(ml_dtypes.bfloat16))
    wvT = np.ascontiguousarray(np.asarray(inputs["Wv"], dtype=f32).T.astype(ml_dtypes.bfloat16))
    wgT = np.ascontiguousarray(np.asarray(inputs["Wg"], dtype=f32).T.astype(ml_dtypes.bfloat16))
    woT = np.ascontiguousarray(np.asarray(inputs["Wo"], dtype=f32).T.astype(ml_dtypes.bfloat16))
    bg = np.ascontiguousarray(np.asarray(inputs["bg"], dtype=f32))

    # biasT[h, k, q] = bias[h, q, k], in bf16 (bias is O(1); bf16 rounding
    # perturbs scores by ~2^-9 — well inside tolerance)
    biasT_shared = np.ascontiguousarray(
        bias.transpose(0, 2, 1).astype(ml_dtypes.bfloat16))

    B = q.shape[0]
    in_maps = []
    for b in range(B):
        if np.any(mask[b]):
            # additive mask is per-(batch, k): per-partition constant in the
            # transposed layout
            biasT_b = np.ascontiguousarray(
                (bias.transpose(0, 2, 1) + mask[b].reshape(1, S, 1))
                .astype(ml_dtypes.bfloat16))
        else:
            biasT_b = biasT_shared
        bf16 = ml_dtypes.bfloat16
        in_maps.append({
            "qT": np.ascontiguousarray(q[b].T.astype(bf16)),
            "kT": np.ascontiguousarray(k[b].T.astype(bf16)),
            "vT": np.ascontiguousarray(v[b].T.astype(bf16)),
            "biasT": biasT_b,
            "WqT": wqT, "WkT": wkT, "WvT": wvT, "WgT": wgT, "WoT": woT,
            "bg": bg,
        })
    res = run_bass_kernel_spmd(nc, in_maps, list(range(NCORES)),
                               trace=trace, **spmd_kwargs)
    out = np.stack([res.results[i]["out"] for i in range(NCORES)], axis=0)
    return out, res


def kernel(**inputs) -> np.ndarray:
    out, _ = run(inputs)
    return out.astype(np.float32)
